# revision 8
# baseline (speedup 1.0000x reference)
"""Multi-head causal attention on 8 Trainium2 NeuronCores.

Problem: B=2, T=2048, C=1024, H=16, HS=64 (fp32), causal mask.

Sharding: 8 cores = 2 batches x 4 head-groups (4 heads each). Each core
computes q/k/v projections + attention + its partial output projection for
its 4 heads of its batch; the host sums the 4 per-batch partials (the
all-reduce of the tensor-parallel output projection) and adds the bias.

Per-core kernel dataflow (everything "transposed", T on the free axis):
  qT/kT [heads(64)x2, T] = W.T @ xT          (PE, K=C chunks of 128)
  v     [T, 64+ones]                         (PE)
  sT    [ts=128, tq=512] = kT.T-slice @ qT   (PE)  -> exp(s/8) (ACT)
  causal: multiplicative 0/1 mask tiles on the 4 diagonal ts-chunks (DVE)
  attnT_aug [65, tq] += v_aug.T @ expT       (PE, ones column => row 64 = softmax denom)
  recip = 1/denom (DVE), broadcast over 64 partitions via K=1 matmul (PE)
  attnT = attnT_aug[0:64] * recip            (DVE)  (odd head -> partition-shift DMA)
  y_partial [tq, C] = attnT_pair.T @ wproj   (PE, K=128 per head-pair)

float32r = full-precision fp32 matmul at 1 cycle/row (vs 4 for plain fp32).
"""

import numpy as np

B, T, C, H, HS = 2, 2048, 1024, 16, 64
NCORES = 8
HPC = 4            # heads per core
NKC = C // 128     # contraction chunks (8)
NJ = T // 512      # tq chunks (4)
NTS = T // 128     # ts chunks (16)

_NC_CACHE = {}


def _build_nc():
    if "nc" in _NC_CACHE:
        return _NC_CACHE["nc"]
    from contextlib import ExitStack
    import concourse.bass as bass
    from concourse import bacc, tile, mybir

    f32 = mybir.dt.float32
    f32r = mybir.dt.float32r
    EXP = mybir.ActivationFunctionType.Exp

    nc = bacc.Bacc("TRN2", target_bir_lowering=False, debug=False,
                   enable_asserts=False, num_devices=NCORES)

    xT_d = nc.dram_tensor("xT", (C, T), f32, kind="ExternalInput").ap()
    wq_d = nc.dram_tensor("wq_s", (C, HPC * HS), f32, kind="ExternalInput").ap()
    wk_d = nc.dram_tensor("wk_s", (C, HPC * HS), f32, kind="ExternalInput").ap()
    wv_d = nc.dram_tensor("wv_s", (C, HPC * HS), f32, kind="ExternalInput").ap()
    wp_d = nc.dram_tensor("wp_s", (HPC * HS, C), f32, kind="ExternalInput").ap()
    mask_d = nc.dram_tensor("mask01", (4, 128, 512), f32, kind="ExternalInput").ap()
    y_d = nc.dram_tensor("y", (T, C), f32, kind="ExternalOutput").ap()

    scale = float(1.0 / np.sqrt(HS))

    with tile.TileContext(nc) as tc, ExitStack() as ctx:
        persist = ctx.enter_context(tc.tile_pool(name="persist", bufs=1))
        work = ctx.enter_context(tc.tile_pool(name="work", bufs=3))
        small = ctx.enter_context(tc.tile_pool(name="small", bufs=2))
        outp = ctx.enter_context(tc.tile_pool(name="outp", bufs=3))
        psp = ctx.enter_context(tc.tile_pool(name="psp", bufs=2, space="PSUM"))
        psatt = ctx.enter_context(tc.tile_pool(name="psatt", bufs=2, space="PSUM"))
        psy = ctx.enter_context(tc.tile_pool(name="psy", bufs=2, space="PSUM"))

        # ---- persistent SBUF tensors (f32r = fast-fp32 PE path, ~1.6e-4) ----
        xt = [persist.tile([128, T], f32r, tag=f"xt{c}", name=f"xt{c}") for c in range(NKC)]
        wq_sb = persist.tile([128, NKC, 256], f32r, tag="wq")
        wk_sb = persist.tile([128, NKC, 256], f32r, tag="wk")
        wv_sb = persist.tile([128, NKC, 256], f32r, tag="wv")
        wp_sb = persist.tile([128, 2, C], f32r, tag="wp")
        mask_sb = persist.tile([128, 4, 512], f32r, tag="mask")
        qT = [persist.tile([128, T], f32r, tag=f"qT{p}", name=f"qT{p}") for p in range(2)]
        kT = [persist.tile([128, T], f32r, tag=f"kT{p}", name=f"kT{p}") for p in range(2)]
        vt = [persist.tile([128, NTS * 65], f32r, tag=f"vt{h}", name=f"vt{h}") for h in range(HPC)]
        attnT = [persist.tile([128, T], f32r, tag=f"attnT{p}", name=f"attnT{p}") for p in range(2)]

        # ---- loads ----
        for c in range(NKC):
            nc.sync.dma_start(out=xt[c], in_=xT_d[c * 128:(c + 1) * 128, :].bitcast(f32r))
        nc.sync.dma_start(out=wq_sb, in_=wq_d.rearrange("(c p) m -> p c m", p=128).bitcast(f32r))
        nc.sync.dma_start(out=wk_sb, in_=wk_d.rearrange("(c p) m -> p c m", p=128).bitcast(f32r))
        nc.sync.dma_start(out=wv_sb, in_=wv_d.rearrange("(c p) m -> p c m", p=128).bitcast(f32r))
        nc.sync.dma_start(out=wp_sb, in_=wp_d.rearrange("(k p) n -> p k n", p=128).bitcast(f32r))
        nc.sync.dma_start(out=mask_sb, in_=mask_d.rearrange("d p f -> p d f").bitcast(f32r))

        # ---- q/k projections: qT[pair] rows 64*hh -> head 2*pair+hh ----
        # two tq-chunks (J, J+1) share one 2-bank psum tile -> one wide copy
        for pair in range(2):
            for dst, w_sb in ((qT[pair], wq_sb), (kT[pair], wk_sb)):
                for Jp in range(NJ // 2):
                    ps = psp.tile([128, 1024], f32, tag="s")
                    for half in range(2):
                        J = 2 * Jp + half
                        for c in range(NKC):
                            nc.tensor.matmul(
                                ps[:, 512 * half:512 * half + 512],
                                lhsT=w_sb[:, c, 128 * pair:128 * pair + 128],
                                rhs=xt[c][:, 512 * J:512 * J + 512],
                                start=(c == 0), stop=(c == NKC - 1))
                    nc.vector.tensor_copy(
                        out=dst[:, 1024 * Jp:1024 * Jp + 1024], in_=ps)

        # ---- v: all 4 heads at once (N=256), strided into vt[h] [T, 65] ----
        for t in range(NTS):
            ps = psp.tile([128, 1024], f32, tag="s")
            for c in range(NKC):
                nc.tensor.matmul(
                    ps[:, 0:256],
                    lhsT=xt[c][:, 128 * t:128 * t + 128],
                    rhs=wv_sb[:, c, :],
                    start=(c == 0), stop=(c == NKC - 1))
            for h in range(HPC):
                nc.vector.tensor_copy(
                    out=vt[h][:, 65 * t:65 * t + 64], in_=ps[:, 64 * h:64 * h + 64])
        ones16 = persist.tile([128, NTS, 1], f32, tag="ones16")
        nc.vector.memset(ones16, 1.0)
        for h in range(HPC):
            nc.vector.tensor_copy(
                out=vt[h].rearrange("p (t x) -> p t x", x=65)[:, :, 64:65],
                in_=ones16)

        # ---- attention ----
        for pair in range(2):
            for hh in range(2):
                h = 2 * pair + hh
                for J in range(NJ):
                    nch = 4 * J + 4
                    pa = psatt.tile([65, 512], f32, tag="att")
                    for u in range(nch // 2):
                        t0, t1 = 2 * u, 2 * u + 1
                        ss = psp.tile([128, 1024], f32, tag="s")
                        for half, t in ((0, t0), (1, t1)):
                            nc.tensor.matmul(
                                ss[:, 512 * half:512 * half + 512],
                                lhsT=kT[pair][64 * hh:64 * hh + 64,
                                              128 * t:128 * t + 128],
                                rhs=qT[pair][64 * hh:64 * hh + 64,
                                             512 * J:512 * J + 512],
                                start=True, stop=True)
                        et = work.tile([128, 1024], f32r, tag="et")
                        nc.scalar.activation(out=et, in_=ss, func=EXP, scale=scale)
                        for half, t in ((0, t0), (1, t1)):
                            if t >= 4 * J:
                                sl = et[:, 512 * half:512 * half + 512]
                                nc.vector.tensor_mul(sl, sl, mask_sb[:, t - 4 * J, :])
                        for half, t in ((0, t0), (1, t1)):
                            nc.tensor.matmul(
                                pa,
                                lhsT=vt[h][:, 65 * t:65 * t + 65],
                                rhs=et[:, 512 * half:512 * half + 512],
                                start=(t == 0), stop=(t == nch - 1))
                    recip = small.tile([1, 512], f32, tag="recip")
                    nc.vector.reciprocal(recip, pa[64:65, :])
                    bcast = small.tile([64, 512], f32, tag="bcast")
                    nc.gpsimd.partition_broadcast(bcast, recip)
                    if hh == 0:
                        nc.vector.tensor_mul(
                            attnT[pair][0:64, 512 * J:512 * J + 512],
                            pa[0:64, :], bcast)
                    else:
                        tmp = small.tile([64, 512], f32r, tag="tmp")
                        nc.vector.tensor_mul(tmp, pa[0:64, :], bcast)
                        nc.sync.dma_start(
                            out=attnT[pair][64:128, 512 * J:512 * J + 512], in_=tmp)

        # ---- output projection (partial: this core's 4 heads) ----
        for m in range(T // 128):
            for n in range(2):
                py_ = psy.tile([128, 512], f32, tag="y")
                for pair in range(2):
                    nc.tensor.matmul(
                        py_,
                        lhsT=attnT[pair][:, 128 * m:128 * m + 128],
                        rhs=wp_sb[:, pair, 512 * n:512 * n + 512],
                        start=(pair == 0), stop=(pair == 1))
                yo = outp.tile([128, 512], f32, tag="yo")
                nc.vector.tensor_copy(out=yo, in_=py_)
                nc.sync.dma_start(
                    out=y_d[128 * m:128 * m + 128, 512 * n:512 * n + 512], in_=yo)

    nc.compile()
    _NC_CACHE["nc"] = nc
    return nc


def _make_mask01():
    m = np.zeros((4, 128, 512), dtype=np.float32)
    p = np.arange(128)[:, None]
    f = np.arange(512)[None, :]
    for d in range(4):
        m[d] = (f >= 128 * d + p).astype(np.float32)
    return m


def make_in_maps(x, wq, wk, wv, wproj):
    mask01 = _make_mask01()
    xTs = [np.ascontiguousarray(x[b].T) for b in range(B)]
    in_maps = []
    for core in range(NCORES):
        b, g = divmod(core, 4)
        hs = slice(4 * g, 4 * g + 4)
        in_maps.append({
            "xT": xTs[b],
            "wq_s": np.ascontiguousarray(wq[hs].transpose(1, 0, 2).reshape(C, HPC * HS)),
            "wk_s": np.ascontiguousarray(wk[hs].transpose(1, 0, 2).reshape(C, HPC * HS)),
            "wv_s": np.ascontiguousarray(wv[hs].transpose(1, 0, 2).reshape(C, HPC * HS)),
            "wp_s": np.ascontiguousarray(wproj[4 * g * HS:(4 * g + 4) * HS, :]),
            "mask01": mask01,
        })
    return in_maps


def _assemble(results, bproj):
    y = np.zeros((B, T, C), dtype=np.float32)
    for core in range(NCORES):
        y[core // 4] += results[core]["y"]
    y += bproj.astype(np.float32)[None, None, :]
    return y


def _is_causal(attention_mask):
    tril = np.tril(np.ones((T, T), dtype=bool))
    return all(np.array_equal(attention_mask[b], tril) for b in range(B))


def _numpy_fallback(x, attention_mask, wq, wk, wv, wproj, bproj):
    x64 = x.astype(np.float32)
    q = np.einsum('btc,hcd->bhtd', x64, wq)
    k = np.einsum('btc,hcd->bhtd', x64, wk)
    v = np.einsum('btc,hcd->bhtd', x64, wv)
    wei = np.einsum('bhtd,bhsd->bhts', q, k) / np.sqrt(np.float32(HS))
    wei = np.where(attention_mask[:, None, :, :], wei, -np.inf)
    wei = wei - wei.max(axis=-1, keepdims=True)
    wei = np.exp(wei)
    wei = wei / wei.sum(axis=-1, keepdims=True)
    out = np.einsum('bhts,bhsd->bhtd', wei, v)
    out = out.transpose(0, 2, 1, 3).reshape(B, T, H * HS)
    return (out @ wproj + bproj).astype(np.float32)


def _install_ntff_hook():
    """Recreate the antenv.axon_hooks shim so trace=True works under axon."""
    import sys, types
    try:
        from antenv.axon_hooks import get_axon_ntff_profile_hook  # noqa
        return
    except ImportError:
        pass
    import antenv
    mod = types.ModuleType("antenv.axon_hooks")
    holder = [None]
    mod.set_axon_ntff_profile_hook = lambda h: holder.__setitem__(0, h)
    mod.get_axon_ntff_profile_hook = lambda: holder[0]
    sys.modules["antenv.axon_hooks"] = mod
    antenv.axon_hooks = mod
    if "/root/.axon_site" not in sys.path:
        sys.path.insert(0, "/root/.axon_site")
    from trn_agent_boot.trn_boot import _ntff_profile_via_ctypes
    mod.set_axon_ntff_profile_hook(_ntff_profile_via_ctypes("/opt/axon/libaxon_pjrt.so"))


def kernel(x, attention_mask, wq, wk, wv, wproj, bproj, _trace=False):
    x = np.asarray(x); attention_mask = np.asarray(attention_mask)
    wq = np.asarray(wq); wk = np.asarray(wk); wv = np.asarray(wv)
    wproj = np.asarray(wproj); bproj = np.asarray(bproj)

    if not _is_causal(attention_mask):
        return _numpy_fallback(x, attention_mask, wq, wk, wv, wproj, bproj)

    from concourse import bass_utils
    if _trace:
        _install_ntff_hook()
        bass_utils.upload_artifacts = lambda d: d
    nc = _build_nc()
    in_maps = make_in_maps(x, wq, wk, wv, wproj)
    res = bass_utils.run_bass_kernel_spmd(
        nc, in_maps, core_ids=list(range(NCORES)), trace=_trace)
    out = _assemble(res.results, bproj)
    if _trace:
        return out, res
    return out
